# revision 58
# baseline (speedup 1.0000x reference)
"""Multi-head self-attention Trainium2 kernel (8 NeuronCores).

Sharding: 8 cores = 4 batches x 2 head-groups (8 heads each).
Core c handles batch b=c//2, heads [g*8, (g+1)*8) where g=c%2.
Each core computes a partial output (its heads' contribution to the
output projection); the host sums the two partials per batch and adds bo.

All matmuls run in float32r (fp32 data, ~1 cycle/row vs 4 for fp32,
~1.5e-4 matmul rel err). fp32r matmuls require output base partition 0.

Per-core dataflow:
  xT [1024, 2048] (= x[b].T), wq/wk/wv [1024, 512], wo [512, 1024]
  A1: QT[p]/KT[p] = w_p.T @ x.T  [128, 2048] per head-pair p (2 heads x 64
      dims on partitions). PSUM accum over 8 k-tiles.
  A2: VS[jt] = [x_jt @ wv | ones] per 128-token tile: [128, 8*65] with a
      ones column per head (the ones column makes the PV matmul emit the
      softmax normalizer as row 64 of the context tile).
  B:  per (pair p, 512-query block qb):
        ST[j-tile, i] = KT_h-slice.T x QT_h  (K=64, head pair row-packed)
        PT = exp(0.125 * ST)                 (ScalarE, 1536/1024-elem groups)
        ct_par[c(65), i] += VS[jt]_h.T @ PT  (row 64 accumulates sum(exp))
      normalize (kept off the PE critical path so HAM stays warm):
        rc = approx-recip(ct row 64) (1 fast DVE op); ctx copied out of
        PSUM immediately (frees the accumulator bank for the next pair);
        K=1 matmul broadcasts rc back into the freed PSUM region; DVE
        mult ctx*rc -> cth[h] [64, 512] per head (fp32r)
  C:  per qb: out[tokens, :] = sum_h cth[h].T-slice @ wo_h  (K=64 accum)
"""

import numpy as np
import ml_dtypes
from collections import deque

import concourse.bass as bass
import concourse.tile as tile
from concourse import bacc, mybir
from contextlib import ExitStack

P = 128
D = 1024
HD = 512  # head dims per core (8 heads x 64)
NPAIR = 4
NH = 8
F32 = mybir.dt.float32
FR = mybir.dt.float32r
BF = mybir.dt.bfloat16


def _st_groups(n_slices):
    """Split n_slices exp slices into alternating groups of 3 and 2."""
    groups = []
    want = 3
    rem = n_slices
    while rem > 0:
        g = min(want, rem)
        groups.append(g)
        rem -= g
        want = 2 if want == 3 else 3
    return groups


def build_nc(S=2048):
    NKT = D // P          # 8 k-tiles over model dim
    NJT = S // P          # key tiles
    MSEG = 512
    NMSEG = S // MSEG
    QB = 512
    NQB = S // QB

    nc = bacc.Bacc("TRN2", target_bir_lowering=False, debug=False)
    xT = nc.dram_tensor("xT", [D, S], BF, kind="ExternalInput").ap()
    wq = nc.dram_tensor("wq", [D, HD], BF, kind="ExternalInput").ap()
    wk = nc.dram_tensor("wk", [D, HD], BF, kind="ExternalInput").ap()
    wv = nc.dram_tensor("wv", [D, HD], BF, kind="ExternalInput").ap()
    wo = nc.dram_tensor("wo", [HD, D], BF, kind="ExternalInput").ap()
    out = nc.dram_tensor("out", [S, D], F32, kind="ExternalOutput").ap()

    with tile.TileContext(nc) as tc:
        with ExitStack() as persist:
            const_pool = persist.enter_context(tc.tile_pool(name="const", bufs=1))
            data_pool = persist.enter_context(tc.tile_pool(name="data", bufs=1))

            ones_f32 = const_pool.tile([P, 64], F32, tag="ones32", name="ones_f32")
            nc.vector.memset(ones_f32[:], 1.0)
            ones = const_pool.tile([P, 64], FR, tag="ones", name="ones")
            nc.vector.tensor_copy(ones[:], ones_f32[:])
            ones8_f32 = const_pool.tile([P, NH], F32, tag="ones8", name="ones8_f32")
            nc.vector.memset(ones8_f32[:], 1.0)

            QT = [data_pool.tile([P, S], BF, tag=f"qt{p}", name=f"qt{p}")
                  for p in range(NPAIR)]
            KT = [data_pool.tile([P, S], BF, tag=f"kt{p}", name=f"kt{p}")
                  for p in range(NPAIR)]
            # [128 tokens, 8 heads x (64 dims + ones col)]
            VS = [data_pool.tile([P, NH * 65], BF, tag=f"vs{j}", name=f"vs{j}")
                  for j in range(NJT)]

            # ---------------- Phase A: projections ----------------
            with ExitStack() as es_a:
                w_pool = es_a.enter_context(tc.tile_pool(name="wpool", bufs=1))
                chunk_pool = es_a.enter_context(tc.tile_pool(name="chunks", bufs=12))

                # per-kt weight tiles: the first matmul only waits on the
                # first 256KB slice, and the DMAs interleave with the first
                # mseg's x chunks instead of queueing all 6MB up front.
                wqt, wkt, wvt = [], [], []
                for kt in range(NKT):
                    wqt.append(w_pool.tile([P, HD], BF, tag=f"wq{kt}",
                                           name=f"wq{kt}"))
                    wkt.append(w_pool.tile([P, HD], BF, tag=f"wk{kt}",
                                           name=f"wk{kt}"))
                    wvt.append(w_pool.tile([P, HD], BF, tag=f"wv{kt}",
                                           name=f"wv{kt}"))

                # --- A1: QT / KT (8 PSUM accumulators: (q|k) x 4 pairs) ---
                with tc.tile_pool(name="qkps", bufs=8, space="PSUM") as qk_pool:
                    for mseg in range(NMSEG):
                        accs = [qk_pool.tile([P, MSEG], F32, tag="qk", name="qkacc")
                                for _ in range(8)]
                        for kt in range(NKT):
                            if mseg == 0:
                                nc.sync.dma_start(
                                    wqt[kt][:], wq[kt * P:(kt + 1) * P, :])
                                nc.sync.dma_start(
                                    wkt[kt][:], wk[kt * P:(kt + 1) * P, :])
                            xc = chunk_pool.tile([P, MSEG], BF, tag="xc", name="xc")
                            nc.sync.dma_start(
                                xc[:],
                                xT[kt * P:(kt + 1) * P, mseg * MSEG:(mseg + 1) * MSEG])
                            for p in range(NPAIR):
                                for ti, wt in ((0, wqt), (1, wkt)):
                                    nc.tensor.matmul(
                                        accs[p * 2 + ti][:],
                                        lhsT=wt[kt][:, p * P:(p + 1) * P],
                                        rhs=xc[:],
                                        start=(kt == 0), stop=(kt == NKT - 1))
                        for p in range(NPAIR):
                            nc.scalar.copy(
                                QT[p][:, mseg * MSEG:(mseg + 1) * MSEG], accs[p * 2][:])
                            nc.vector.tensor_copy(
                                KT[p][:, mseg * MSEG:(mseg + 1) * MSEG], accs[p * 2 + 1][:])

                # --- A2: V (natural layout, 4 j-tiles per mseg) ---
                with tc.tile_pool(name="vps", bufs=8, space="PSUM") as v_pool:
                    for mseg in range(NMSEG):
                        vaccs = [v_pool.tile([P, HD], F32, tag="v", name="vacc")
                                 for _ in range(4)]
                        for kt in range(NKT):
                            if mseg == 0:
                                nc.sync.dma_start(
                                    wvt[kt][:], wv[kt * P:(kt + 1) * P, :])
                            xc = chunk_pool.tile([P, MSEG], BF, tag="xc", name="xc")
                            nc.sync.dma_start(
                                xc[:],
                                xT[kt * P:(kt + 1) * P, mseg * MSEG:(mseg + 1) * MSEG])
                            for i in range(4):
                                nc.tensor.matmul(
                                    vaccs[i][:],
                                    lhsT=xc[:, i * P:(i + 1) * P],
                                    rhs=wvt[kt][:],
                                    start=(kt == 0), stop=(kt == NKT - 1))
                        for i in range(4):
                            vsv = VS[mseg * 4 + i].rearrange("p (h c) -> p h c", c=65)
                            if i % 2 == 0:
                                nc.scalar.copy(vsv[:, :, 0:64], vaccs[i][:])
                            else:
                                nc.vector.tensor_copy(vsv[:, :, 0:64], vaccs[i][:])
                            nc.vector.tensor_copy(vsv[:, :, 64], ones8_f32[:])

            # ---------------- Phases B + C: attention + projection ----------------
            with ExitStack() as es_b:
                cth_pool = es_b.enter_context(tc.tile_pool(name="cthpool", bufs=3))
                ctx_pool = es_b.enter_context(tc.tile_pool(name="ctxpool", bufs=6))
                wo_pool = es_b.enter_context(tc.tile_pool(name="wopool", bufs=1))
                pt_pool = es_b.enter_context(tc.tile_pool(name="ptpool", bufs=6))
                rc_pool = es_b.enter_context(tc.tile_pool(name="rcpool", bufs=6))
                po_pool = es_b.enter_context(tc.tile_pool(name="popool", bufs=4))
                st_ps = es_b.enter_context(tc.tile_pool(name="stps", bufs=1, space="PSUM"))
                ct_ps = es_b.enter_context(tc.tile_pool(name="ctps", bufs=1, space="PSUM"))
                pj_ps = es_b.enter_context(tc.tile_pool(name="pjps", bufs=1, space="PSUM"))

                # wo packed per head-pair: [128, 1024] so proj runs K=128
                wo_h = []
                for pp in range(NPAIR):
                    t = wo_pool.tile([P, D], BF, tag=f"wo{pp}", name=f"wo{pp}")
                    nc.sync.dma_start(t[:], wo[pp * P:(pp + 1) * P, :])
                    wo_h.append(t)

                groups = _st_groups(2 * NJT)
                stages = [(qb, p) for qb in range(NQB) for p in range(NPAIR)]
                NST = len(stages)
                cth = {}
                ctss = {}

                def emit_pv(i, ptg, s0, gl):
                    _, p = stages[i]
                    cts = ctss[i]
                    for l in range(gl):
                        s = s0 + l
                        jt, par = divmod(s, 2)
                        h = 2 * p + par
                        nc.tensor.matmul(
                            cts[par][:],
                            lhsT=VS[jt][:, h * 65:(h + 1) * 65],
                            rhs=ptg[:, l * 512:(l + 1) * 512],
                            start=(jt == 0), stop=(jt == NJT - 1))

                def norm_start(i):
                    # Drain the accumulators to SBUF (frees the ct PSUM
                    # banks) and kick off the reciprocals (DVE-only, the PE
                    # is not involved). Both ctx copies go first: the DVE is
                    # FIFO, so a recip before the second copy would delay
                    # that ct bank's release by ~3.3us.
                    qb, p = stages[i]
                    pre = []
                    for par in range(2):
                        ct = ctss[i][par]
                        ctx = ctx_pool.tile([65, QB], F32, tag="ctx", name="ctx")
                        nc.vector.tensor_copy(ctx[:], ct[:])
                        pre.append(ctx)
                    for par in range(2):
                        ctx = pre[par]
                        rc = rc_pool.tile([65, QB], FR, tag="rc", name="rc")
                        with nc.allow_low_precision(reason="softmax recip"):
                            nc.vector.reciprocal(rc[64:65, :], ctx[64:65, :])
                        cth[(i, par, "pre")] = (ctx, rc)
                    del ctss[i]

                def finish_normalize(i):
                    # bc matmul + mult for a pair whose recip was issued a
                    # stage ago (so the PE never waits on the reciprocal).
                    # Both heads land in one [128, 512] tile (K=128 proj):
                    # par0 via the DVE mult directly, par1 multiplied at
                    # partitions 0-63 then DMA-relocated to partitions 64-127
                    # (DVE lanes cannot cross partitions; DMA can).
                    qb, p = stages[i]
                    pair_t = cth_pool.tile([P, QB], BF, tag=f"cth{p}",
                                           name=f"cth{p}")
                    # par1 first: its extra DMA relocation starts earlier
                    for par in (1, 0):
                        ctx, rc = cth[(i, par, "pre")]
                        bc = pj_ps.tile([64, QB], F32, tag="pj", name="bc")
                        nc.tensor.matmul(
                            bc[:], lhsT=ones[64:65, :],
                            rhs=rc[64:65, :], start=True, stop=True)
                        if par == 0:
                            nc.vector.tensor_tensor(
                                pair_t[0:64, :], bc[:], ctx[0:64, :],
                                mybir.AluOpType.mult)
                        else:
                            tmp = ctx_pool.tile([64, QB], BF, tag="nrm1",
                                                name="nrm1")
                            nc.vector.tensor_tensor(
                                tmp[:], bc[:], ctx[0:64, :],
                                mybir.AluOpType.mult)
                            nc.sync.dma_start(pair_t[64:128, :], tmp[:])
                        del cth[(i, par, "pre")]
                    cth[(qb, p)] = pair_t

                proj_q = deque()
                proj_left = {}

                def emit_po_block(qb, mtl, half, pool_tag=None):
                    # one output tile of phase C: K=128, two heads per matmul.
                    # The PSUM drain runs on ScalarE, which has idle slots in
                    # phase B (the DVE is busy with reciprocals).
                    pool, tag = pool_tag or (pj_ps, "pj")
                    mt = qb * 4 + mtl
                    po = pool.tile([P, 512], F32, tag=tag, name="po")
                    for pp in range(NPAIR):
                        nc.tensor.matmul(
                            po[:],
                            lhsT=cth[(qb, pp)][:, mtl * P:(mtl + 1) * P],
                            rhs=wo_h[pp][:, half * 512:(half + 1) * 512],
                            start=(pp == 0), stop=(pp == NPAIR - 1))
                    po_sb = po_pool.tile([P, 512], F32, tag="posb", name="po_sb")
                    nc.scalar.copy(po_sb[:], po[:])
                    nc.sync.dma_start(
                        out[mt * P:(mt + 1) * P, half * 512:(half + 1) * 512],
                        po_sb[:])
                    proj_left[qb] -= 1
                    if proj_left[qb] == 0:
                        for pp in range(NPAIR):
                            del cth[(qb, pp)]

                def enqueue_proj(qb):
                    proj_left[qb] = 8
                    for mtl in range(4):
                        for half in range(2):
                            proj_q.append((qb, mtl, half))

                def drain_proj(n, pool_tag=None):
                    for _ in range(min(n, len(proj_q))):
                        emit_po_block(*proj_q.popleft(), pool_tag=pool_tag)

                # Cross-stage software pipeline. The PE queue is in-order:
                # any emitted instruction that waits (PV on its exp, bc on
                # its recip) blocks everything behind it. So PV trails ST by
                # one exp-group ACROSS stage boundaries, the normalize for
                # stage i-1 and the bc/mult/proj for stage i-2 are emitted
                # inside stage i's ST/exp stream, where their inputs are
                # long since ready.
                pv_q = deque()
                for i, (qb, p) in enumerate(stages):
                    s0 = 0
                    for gi, gl in enumerate(groups):
                        tag = "stA" if gl == 3 else "stB"
                        stg = st_ps.tile([P, gl * 512], F32, tag=tag, name="stg")
                        for l in range(gl):
                            s = s0 + l
                            jt, par = divmod(s, 2)
                            nc.tensor.matmul(
                                stg[:, l * 512:(l + 1) * 512],
                                lhsT=KT[p][par * 64:(par + 1) * 64,
                                           jt * P:(jt + 1) * P],
                                rhs=QT[p][par * 64:(par + 1) * 64,
                                          qb * QB:(qb + 1) * QB],
                                start=True, stop=True)
                        ptg = pt_pool.tile([P, gl * 512], BF, tag=tag, name="ptg")
                        nc.scalar.activation(
                            ptg[:], stg[:],
                            mybir.ActivationFunctionType.Exp, scale=0.125)
                        if gi == 3:
                            # allocate this stage's accumulators only after
                            # the previous stage's were drained (ct bufs=1)
                            ctss[i] = [
                                ct_ps.tile([65, QB], F32, tag="cte", name="cte"),
                                ct_ps.tile([65, QB], F32, tag="cto", name="cto")]
                        npop = 0
                        while len(pv_q) >= 3 and npop < 2:
                            emit_pv(*pv_q.popleft())
                            npop += 1
                        if gi == 1 and i >= 2:
                            finish_normalize(i - 2)
                            pqb, pp = stages[i - 2]
                            if pp == NPAIR - 1:
                                enqueue_proj(pqb)
                        if gi == 2 and i >= 1:
                            # recips: the long-latency item the NEXT stage's
                            # bc matmuls wait on (after the g12 pop above)
                            norm_start(i - 1)
                        if gi == 4:
                            drain_proj(1)
                        if gi == 8:
                            drain_proj(1)
                            if i == NST - 1:
                                # last stage: pull in the previous pair's
                                # finish so the epilogue chain is shorter
                                finish_normalize(i - 1)
                        pv_q.append((i, ptg, s0, gl))
                        s0 += gl

                # epilogue: drain the pipeline
                while pv_q:
                    emit_pv(*pv_q.popleft())
                norm_start(NST - 1)
                finish_normalize(NST - 1)
                enqueue_proj(NQB - 1)
                epools = [(pj_ps, "pj"), (st_ps, "stA"), (st_ps, "stB")]
                k = 0
                while proj_q:
                    drain_proj(1, pool_tag=epools[k % 3])
                    k += 1
    nc.compile()
    return nc


_NC_CACHE = {}


def _get_nc(S=2048):
    if S not in _NC_CACHE:
        _NC_CACHE[S] = build_nc(S)
    return _NC_CACHE[S]


def kernel(x, Wq, Wk, Wv, Wo, bo):
    from concourse.bass_utils import run_bass_kernel_spmd

    x = np.asarray(x, dtype=np.float32)
    Wq = np.asarray(Wq, dtype=np.float32)
    Wk = np.asarray(Wk, dtype=np.float32)
    Wv = np.asarray(Wv, dtype=np.float32)
    Wo = np.asarray(Wo, dtype=np.float32)
    bo = np.asarray(bo, dtype=np.float32)

    bs, S, d = x.shape
    nc = _get_nc(S)

    in_maps = []
    for c in range(8):
        b, g = divmod(c, 2)
        cols = slice(g * HD, (g + 1) * HD)
        bf = ml_dtypes.bfloat16
        in_maps.append({
            "xT": np.ascontiguousarray(x[b].T).astype(bf),
            "wq": np.ascontiguousarray(Wq[:, cols]).astype(bf),
            "wk": np.ascontiguousarray(Wk[:, cols]).astype(bf),
            "wv": np.ascontiguousarray(Wv[:, cols]).astype(bf),
            "wo": np.ascontiguousarray(Wo[cols, :]).astype(bf),
        })

    res = run_bass_kernel_spmd(nc, in_maps, core_ids=list(range(8)))
    outp = np.empty((bs, S, d), dtype=np.float32)
    for b in range(bs):
        outp[b] = res.results[2 * b]["out"] + res.results[2 * b + 1]["out"] + bo
    return outp
